# revision 24
# baseline (speedup 1.0000x reference)
"""Trainium2 Bass kernel for nn_MixedConvWithReLU (moe_routing).

Forward pass of: softmax(alphas)-weighted sum of 3 quantized conv branches
(1/4/16-bit weight quant, 3x3 conv s=1 p=1, BN eval, ActQuant), on
x[32,256,56,56].

Strategy (8 NeuronCores, data-parallel over batch, 4 images/core).
Default variant "winov" = 1D Winograd F(2,3) along W, fp16 operands:

- dtypes: x and weights are fp16 (mixed 16/32-bit matmuls are rejected by
  the compiler; fp8 blows the 2e-2 gate via act-quant boundary flips; fp16's
  10-bit mantissa costs only ~sqrt(2) more flip noise than f32r). Weight
  contents are EXACT where act-quant is sensitive:
    br0 (1-bit):  sign(W) in {-1,0,+1}; scale*inv folded into the threshold.
    br1 (4-bit):  round(W/step) ints in [-7,7]; step*inv*15 applied via the
                  ACT engine's per-partition scale operand.
    br2 (16-bit): fp16(Wq*inv) (~0.05% rounding; its act-quant is nearly
                  continuous so this stays ~1e-3).
- Winograd per output col pair (2t, 2t+1):
    V = [d0-d2, d1+d2, d2-d1, d1-d3] (DVE, fp16, from the padded image)
    U = [g0, (g0+g1+g2)/2, (g0-g1+g2)/2, g2] (host, exact fp16 halves for
        the sign/int branches)
    m_p accumulate in 4 PSUM tiles over (cin-half, kh): 24 matmuls of
    [128]x[128, 392] per (branch, 14-row group) = 12 MM-cycles/output vs 18
    direct. LDWEIGHTS is fully hidden for fp16 weights (measured 190.7ns/MM
    at N=448 = the streaming floor), so no weight-reuse chunking is needed.
    V is stored flat so every matmul rhs is one contiguous span (strided
    rhs APs measurably stall the PE in large kernels).
    y_even = m0+m1+m2, y_odd = m1-m2-m3 on DVE (m1 copied out via ACT
    first: DVE may read only one PSUM operand per op).
- Act-quant epilogue per parity (all DVE/ACT; gpsimd measured slower):
    1-bit : q = (y > tau0) * w0
    4-bit : t = y*s1 + b15 (ACT per-partition scale), magic-RNE clip via
            (t+C) min (C+15), max C, -C, then a3 = q*(w1/15) + acc
    16-bit: v = min(Relu(y + bias2)*w2, w2); o = a3 + v
  Parities interleave into one SBUF tile (strided engine writes), then ONE
  contiguous DMA per row group — strided DMA-to-DRAM is fatal on this HW.
"""
import numpy as np
import concourse.bacc as bacc
import concourse.tile as tile
import concourse.mybir as mybir
from concourse.bass_utils import run_bass_kernel_spmd

F32 = mybir.dt.float32
F32R = mybir.dt.float32r
FP16 = mybir.dt.float16
I32 = mybir.dt.int32
AF = mybir.ActivationFunctionType
ALU = mybir.AluOpType

N_CORES = 8
B, CIN, COUT, H, W, K = 32, 256, 256, 56, 56, 3
B_PER = B // N_CORES          # 4 images per core
RG = 8                        # rows per matmul tile -> free dim 448
N_RG = H // RG                # 7 row-groups
HP = H + 2                    # padded 58
C_MAGIC = np.float32(1.5 * 2**23)
EPS = 1e-5
BITS = (1, 4, 16)

CHUNKS = {
    "seq": ((0,), (1,), (2,), (3,), (4,), (5,), (6,)),
    "ilv2": ((0, 1), (2, 3), (4, 5), (6,)),
    "ilv4": ((0, 1, 2, 3), (4, 5, 6)),
    "ilv7": ((0, 1, 2, 3, 4, 5, 6),),
}

_cache = {}


def _f32_to_f32r_bits(x: np.ndarray) -> np.ndarray:
    """fp32 -> fp32r storage bits (RNE to 11 explicit mantissa bits, <<12).
    Verified bit-identical to the device tensor_copy conversion."""
    b = np.ascontiguousarray(x, np.float32).view(np.uint32).astype(np.uint64)
    keep = 12
    lsb = (b >> keep) & 1
    half = np.uint64((1 << (keep - 1)) - 1)
    rounded = (b + half + lsb) >> np.uint64(keep)
    return ((rounded << np.uint64(keep)) & np.uint64(0xFFFFFFFF)) \
        .astype(np.uint32).view(np.float32)


def _build(variant="seq", loop=True, strip="none"):
    nc = bacc.Bacc(trn_type="TRN2", debug=False)
    xr = nc.dram_tensor("xr", [B_PER, CIN, H, W], FP16, kind="ExternalInput").ap()
    wr = nc.dram_tensor("wr", [128, 2 * 3 * 3 * 6 * 128], FP16,
                        kind="ExternalInput").ap()
    cst = nc.dram_tensor("cst", [128, 14], F32, kind="ExternalInput").ap()
    iters = nc.dram_tensor("iters", [1, 1], I32, kind="ExternalInput").ap()
    out = nc.dram_tensor("out", [B_PER, COUT, H, W], F32, kind="ExternalOutput").ap()

    with tile.TileContext(nc) as tc:
        with (
            tc.tile_pool(name="fix", bufs=1) as fix,
            tc.tile_pool(name="ps", bufs=8, space="PSUM") as ps,
            tc.tile_pool(name="stage", bufs=3) as stage,
        ):
            wsb = fix.tile([128, 2, 3, 3, 6, 128], FP16, tag="wsb")
            cst_t = fix.tile([128, 14], F32, tag="cst")
            xp = [fix.tile([128, 2, HP, HP], FP16, tag=f"xp{s}", name=f"xp{s}")
                  for s in range(2)]

            nc.sync.dma_start(
                out=wsb[:].rearrange("p h kh kw b m -> p (h kh kw b m)"), in_=wr)
            nc.sync.dma_start(out=cst_t[:], in_=cst)

            if loop:
                tmp = nc.alloc_registers("iters_reg", mybir.ALL_ENGINES)
                nc.regs_load(tmp, iters[0:1, 0:1])
                n_it = nc.snap(tmp, donate=True, min_val=1, max_val=1000000)

            # zero the pad borders once (slots keep zero borders forever;
            # per-image DMAs only write the interior)
            for s in range(2):
                for h in range(2):
                    nc.vector.memset(xp[s][:, h, 0, :], 0.0)
                    nc.vector.memset(xp[s][:, h, HP - 1, :], 0.0)
                    nc.vector.memset(xp[s][:, h, 1:HP - 1, 0], 0.0)
                    nc.vector.memset(xp[s][:, h, 1:HP - 1, HP - 1], 0.0)

            from contextlib import nullcontext
            with (tc.For_i(0, n_it, 1) if loop else nullcontext()):
                if loop:
                    nc.gpsimd.nop()
                for img in range(B_PER):
                    s = img % 2
                    if strip != "pe":
                        for h in range(2):
                            nc.scalar.dma_start(
                                out=xp[s][:, h, 1:H + 1, 1:W + 1],
                                in_=xr[img, 128 * h:128 * (h + 1), :, :])
                    for j in range(2):
                        accs = {}
                        a3s = {}
                        for br in range(3):
                            blk = 2 * br + j
                            for chunk in CHUNKS[variant]:
                                pts = {}
                                for r in chunk:
                                    pts[r] = ps.tile([128, RG, W], F32, tag="ps",
                                                     name=f"pt{r}")
                                n = 0
                                for h in range(2):
                                    for kh in range(3):
                                        for kw in range(3):
                                            for r in chunk:
                                                r0 = RG * r
                                                nc.tensor.matmul(
                                                    out=pts[r][:],
                                                    lhsT=wsb[:, h, kh, kw, blk, :],
                                                    rhs=xp[s][:, h, r0 + kh:r0 + kh + RG,
                                                              kw:kw + W],
                                                    start=(n == 0), stop=(n == 17))
                                            n += 1
                                if strip in ("noepi", "pe"):
                                    continue
                                for r in chunk:
                                    r0 = RG * r
                                    if br == 0:
                                        # 1-bit: acc = (p_sign > tau0) * w0
                                        acc = stage.tile([128, RG, W], F32,
                                                         tag="acc", name="acc", bufs=8)
                                        nc.vector.tensor_scalar(
                                            out=acc[:], in0=pts[r][:],
                                            scalar1=cst_t[:, 0 + j:1 + j],
                                            scalar2=cst_t[:, 6:7],
                                            op0=ALU.is_gt, op1=ALU.mult)
                                        accs[r] = acc
                                    elif br == 1:
                                        # 4-bit: t = p*s1 + b15 (ACT, per-part
                                        # scale); u = min(RNE(t+C), C+15) (DVE);
                                        # q = max(u,C)-C in [0,15] (DVE);
                                        # a3 = q*(w1/15) + acc (DVE stt)
                                        t4 = stage.tile([128, RG, W], F32,
                                                        tag="t4", name="t4")
                                        nc.scalar.activation(
                                            out=t4[:], in_=pts[r][:], func=AF.Identity,
                                            bias=cst_t[:, 2 + j:3 + j],
                                            scale=cst_t[:, 10 + j:11 + j])
                                        u4 = stage.tile([128, RG, W], F32,
                                                        tag="u4", name="u4")
                                        nc.vector.tensor_scalar(
                                            out=u4[:], in0=t4[:],
                                            scalar1=float(C_MAGIC),
                                            scalar2=float(C_MAGIC + 15.0),
                                            op0=ALU.add, op1=ALU.min)
                                        q4 = stage.tile([128, RG, W], F32,
                                                        tag="q4", name="q4")
                                        nc.vector.tensor_scalar(
                                            out=q4[:], in0=u4[:],
                                            scalar1=float(C_MAGIC),
                                            scalar2=float(C_MAGIC),
                                            op0=ALU.max, op1=ALU.subtract)
                                        a3 = stage.tile([128, RG, W], F32,
                                                        tag="a3", name="a3", bufs=8)
                                        nc.vector.scalar_tensor_tensor(
                                            out=a3[:], in0=q4[:], scalar=cst_t[:, 7:8],
                                            in1=accs.pop(r)[:], op0=ALU.mult, op1=ALU.add)
                                        a3s[r] = a3
                                    else:
                                        # 16-bit: y = Relu(p16+bias2);
                                        # v16 = min(y*w2, w2) (DVE);
                                        # o = a3 + v16 -> DMA
                                        y16 = stage.tile([128, RG, W], F32,
                                                         tag="y16", name="y16")
                                        nc.scalar.activation(
                                            out=y16[:], in_=pts[r][:], func=AF.Relu,
                                            bias=cst_t[:, 4 + j:5 + j], scale=1.0)
                                        v16 = stage.tile([128, RG, W], F32,
                                                         tag="v16", name="v16")
                                        nc.vector.tensor_scalar(
                                            out=v16[:], in0=y16[:],
                                            scalar1=cst_t[:, 8:9],
                                            scalar2=cst_t[:, 9:10],
                                            op0=ALU.mult, op1=ALU.min)
                                        o = stage.tile([128, RG, W], F32,
                                                       tag="o", name="o")
                                        nc.vector.tensor_tensor(
                                            out=o[:], in0=a3s.pop(r)[:],
                                            in1=v16[:], op=ALU.add)
                                        if strip == "none":
                                            nc.sync.dma_start(
                                                out=out[img, 128 * j:128 * (j + 1),
                                                        r0:r0 + RG, :],
                                                in_=o[:])
                if loop:
                    nc.gpsimd.nop()
            if strip != "none":
                # outputs must still be written once (timing-only builds)
                nc.sync.dma_start(out=out[0, 0:128, 0, 0:12], in_=cst_t[:, 0:12])

    nc.compile()
    return nc




def _build_flat(loop=True, strip="none"):
    """Contiguous-rhs variant: each image half is a flat 58x58 row-major span
    (pads included). A conv row-group MM reads a CONTIGUOUS 464-elem span
    starting at (r0+kh)*58+kw; psum cols t with t%58 in {56,57} are junk and
    are skipped at DMA-out. This keeps the moving-operand AP 1-level."""
    nc = bacc.Bacc(trn_type="TRN2", debug=False)
    xr = nc.dram_tensor("xr", [B_PER, CIN, H, W], FP16, kind="ExternalInput").ap()
    wr = nc.dram_tensor("wr", [128, 2 * 3 * 3 * 6 * 128], FP16,
                        kind="ExternalInput").ap()
    cst = nc.dram_tensor("cst", [128, 14], F32, kind="ExternalInput").ap()
    iters = nc.dram_tensor("iters", [1, 1], I32, kind="ExternalInput").ap()
    out = nc.dram_tensor("out", [B_PER, COUT, H, W], F32, kind="ExternalOutput").ap()

    NF = 8 * 58              # 464 free elems per matmul
    FL = 58 * 58 + 2         # flat span + 2 tail pad elems

    with tile.TileContext(nc) as tc:
        with (
            tc.tile_pool(name="fix", bufs=1) as fix,
            tc.tile_pool(name="ps", bufs=8, space="PSUM") as ps,
            tc.tile_pool(name="stage", bufs=3) as stage,
        ):
            wsb = fix.tile([128, 2, 3, 3, 6, 128], FP16, tag="wsb")
            cst_t = fix.tile([128, 14], F32, tag="cst")
            xp = [fix.tile([128, 2, FL], FP16, tag=f"xp{s}", name=f"xp{s}")
                  for s in range(2)]

            nc.sync.dma_start(
                out=wsb[:].rearrange("p h kh kw b m -> p (h kh kw b m)"), in_=wr)
            nc.sync.dma_start(out=cst_t[:], in_=cst)

            if loop:
                tmp = nc.alloc_registers("iters_reg", mybir.ALL_ENGINES)
                nc.regs_load(tmp, iters[0:1, 0:1])
                n_it = nc.snap(tmp, donate=True, min_val=1, max_val=1000000)

            # zero everything once; per-image DMAs only write interiors, so
            # pad rows/cols (and the 2-elem tail) stay zero forever
            for s in range(2):
                nc.vector.memset(xp[s][:], 0.0)

            from contextlib import nullcontext
            with (tc.For_i(0, n_it, 1) if loop else nullcontext()):
                if loop:
                    nc.gpsimd.nop()
                for img in range(B_PER):
                    s = img % 2
                    if strip != "pe":
                        for h in range(2):
                            xv = xp[s][:, h, 0:58 * 58].rearrange(
                                "p (r c) -> p r c", c=58)
                            nc.scalar.dma_start(
                                out=xv[:, 1:H + 1, 1:W + 1],
                                in_=xr[img, 128 * h:128 * (h + 1), :, :])
                    for j in range(2):
                        accs = {}
                        a3s = {}
                        for br in range(3):
                            blk = 2 * br + j
                            for r in range(N_RG):
                                r0 = RG * r
                                pt = ps.tile([128, NF], F32, tag="ps", name="pt")
                                n = 0
                                for h in range(2):
                                    for kh in range(3):
                                        for kw in range(3):
                                            base = (r0 + kh) * 58 + kw
                                            nc.tensor.matmul(
                                                out=pt[:],
                                                lhsT=wsb[:, h, kh, kw, blk, :],
                                                rhs=xp[s][:, h, base:base + NF],
                                                start=(n == 0), stop=(n == 17))
                                            n += 1
                                if strip in ("noepi", "pe"):
                                    continue
                                if br == 0:
                                    acc = stage.tile([128, NF], F32,
                                                     tag="acc", name="acc", bufs=8)
                                    nc.vector.tensor_scalar(
                                        out=acc[:], in0=pt[:],
                                        scalar1=cst_t[:, 0 + j:1 + j],
                                        scalar2=cst_t[:, 6:7],
                                        op0=ALU.is_gt, op1=ALU.mult)
                                    accs[r] = acc
                                elif br == 1:
                                    t4 = stage.tile([128, NF], F32,
                                                    tag="t4", name="t4")
                                    nc.scalar.activation(
                                        out=t4[:], in_=pt[:], func=AF.Identity,
                                        bias=cst_t[:, 2 + j:3 + j],
                                        scale=cst_t[:, 10 + j:11 + j])
                                    u4 = stage.tile([128, NF], F32,
                                                    tag="u4", name="u4")
                                    nc.vector.tensor_scalar(
                                        out=u4[:], in0=t4[:],
                                        scalar1=float(C_MAGIC),
                                        scalar2=float(C_MAGIC + 15.0),
                                        op0=ALU.add, op1=ALU.min)
                                    q4 = stage.tile([128, NF], F32,
                                                    tag="q4", name="q4")
                                    nc.vector.tensor_scalar(
                                        out=q4[:], in0=u4[:],
                                        scalar1=float(C_MAGIC),
                                        scalar2=float(C_MAGIC),
                                        op0=ALU.max, op1=ALU.subtract)
                                    a3 = stage.tile([128, NF], F32,
                                                    tag="a3", name="a3", bufs=8)
                                    nc.vector.scalar_tensor_tensor(
                                        out=a3[:], in0=q4[:], scalar=cst_t[:, 7:8],
                                        in1=accs.pop(r)[:], op0=ALU.mult,
                                        op1=ALU.add)
                                    a3s[r] = a3
                                else:
                                    y16 = stage.tile([128, NF], F32,
                                                     tag="y16", name="y16")
                                    nc.scalar.activation(
                                        out=y16[:], in_=pt[:], func=AF.Relu,
                                        bias=cst_t[:, 4 + j:5 + j], scale=1.0)
                                    v16 = stage.tile([128, NF], F32,
                                                     tag="v16", name="v16")
                                    nc.vector.tensor_scalar(
                                        out=v16[:], in0=y16[:],
                                        scalar1=cst_t[:, 8:9],
                                        scalar2=cst_t[:, 9:10],
                                        op0=ALU.mult, op1=ALU.min)
                                    o = stage.tile([128, NF], F32,
                                                   tag="o", name="o")
                                    nc.vector.tensor_tensor(
                                        out=o[:], in0=a3s.pop(r)[:],
                                        in1=v16[:], op=ALU.add)
                                    if strip == "none":
                                        ov = o[:].rearrange(
                                            "p (r c) -> p r c", c=58)
                                        nc.sync.dma_start(
                                            out=out[img, 128 * j:128 * (j + 1),
                                                    r0:r0 + RG, :],
                                            in_=ov[:, :, 0:W])
                if loop:
                    nc.gpsimd.nop()
            if strip != "none":
                nc.sync.dma_start(out=out[0, 0:128, 0, 0:12], in_=cst_t[:, 0:12])

    nc.compile()
    return nc


def _build_wino(loop=True, strip="none", use_gpsimd=True, paired=False,
                fast=False, q4_act=False):
    """1D Winograd F(2,3) along W. For each output col pair (2t, 2t+1):
      V0 = d0-d2, V1 = d1+d2, V2 = d2-d1, V3 = d1-d3   (d_i = xpad[2t+i])
      m_p = sum_{ci,kh} U_p[kh] V_p[.., h+kh, t]        (PE, 4 psum tiles)
      y_even = m0+m1+m2, y_odd = m1-m2-m3               (DVE combines)
    U_p = [g0, (g0+g1+g2)/2, (g0-g1+g2)/2, g2] along kw (host, exact fp16
    for the sign/int branches). 12 MM-cycles/output vs direct's 18.
    Epilogue runs per parity on [128, 14, 28] tiles; DMA-out writes
    strided columns. Epilogue threshold/scale constants are unchanged.
    gpsimd takes the SBUF-only epilogue ops (no PSUM port)."""
    nc = bacc.Bacc(trn_type="TRN2", debug=False)
    xr = nc.dram_tensor("xr", [B_PER, CIN, H, W], FP16, kind="ExternalInput").ap()
    wr = nc.dram_tensor("wr", [128, 2 * 4 * 3 * 6 * 128], FP16,
                        kind="ExternalInput").ap()
    cst = nc.dram_tensor("cst", [128, 14], F32, kind="ExternalInput").ap()
    iters = nc.dram_tensor("iters", [1, 1], I32, kind="ExternalInput").ap()
    out = nc.dram_tensor("out", [B_PER, COUT, H, W], F32, kind="ExternalOutput").ap()

    RGW = 14                  # rows per matmul tile
    NT = 28                   # winograd col-pairs
    NFW = RGW * NT            # 392 free elems per matmul

    with tile.TileContext(nc) as tc:
        with (
            tc.tile_pool(name="fix", bufs=1) as fix,
            tc.tile_pool(name="ps", bufs=8, space="PSUM") as ps,
            tc.tile_pool(name="stage", bufs=4) as stage,
        ):
            usb = fix.tile([128, 2, 4, 3, 6, 128], FP16, tag="usb")
            cst_t = fix.tile([128, 14], F32, tag="cst")
            xp = [fix.tile([128, 2, HP, HP], FP16, tag=f"xp{s}", name=f"xp{s}")
                  for s in range(2)]
            vp = [fix.tile([128, 2, 4, HP * NT], FP16, tag=f"vp{s}",
                           name=f"vp{s}") for s in range(2)]

            nc.sync.dma_start(
                out=usb[:].rearrange("p h q kh b m -> p (h q kh b m)"), in_=wr)
            nc.sync.dma_start(out=cst_t[:], in_=cst)

            if loop:
                tmp = nc.alloc_registers("iters_reg", mybir.ALL_ENGINES)
                nc.regs_load(tmp, iters[0:1, 0:1])
                n_it = nc.snap(tmp, donate=True, min_val=1, max_val=1000000)

            for s in range(2):
                nc.vector.memset(xp[s][:], 0.0)
                nc.vector.memset(vp[s][:], 0.0)

            from contextlib import nullcontext
            with (tc.For_i(0, n_it, 1) if loop else nullcontext()):
                if loop:
                    nc.gpsimd.nop()
                for img in range(B_PER):
                    s = img % 2
                    if strip != "pe":
                        for h in range(2):
                            nc.sync.dma_start(
                                out=xp[s][:, h, 1:H + 1, 1:W + 1],
                                in_=xr[img, 128 * h:128 * (h + 1), :, :])
                        # input transform: d_i = xp cols i, i+2, ..., i+54
                        d = [xp[s][:, :, :, i:i + 55:2] for i in range(4)]
                        vv = [vp[s][:, :, p].rearrange(
                            "p h (r c) -> p h r c", c=NT) for p in range(4)]
                        teng = nc.gpsimd if fast else nc.vector
                        teng.tensor_tensor(
                            out=vv[0], in0=d[0], in1=d[2], op=ALU.subtract)
                        teng.tensor_tensor(
                            out=vv[1], in0=d[1], in1=d[2], op=ALU.add)
                        teng.tensor_tensor(
                            out=vv[2], in0=d[2], in1=d[1], op=ALU.subtract)
                        teng.tensor_tensor(
                            out=vv[3], in0=d[1], in1=d[3], op=ALU.subtract)
                    for j in range(2):
                        accs = {}
                        a3s = {}
                        for br in range(3):
                            blk = 2 * br + j
                            for r in range(4):
                                r0 = RGW * r
                                pts = [ps.tile([128, RGW, NT], F32, tag="ps",
                                               name=f"pt{p}") for p in range(4)]
                                n = 0
                                for h in range(2):
                                    for kh in range(3):
                                        base = (r0 + kh) * NT
                                        for p in range(4):
                                            nc.tensor.matmul(
                                                out=pts[p][:],
                                                lhsT=usb[:, h, p, kh, blk, :],
                                                rhs=vp[s][:, h, p,
                                                          base:base + NFW],
                                                start=(n == 0), stop=(n == 5))
                                        n += 1
                                if strip in ("noepi", "pe"):
                                    continue
                                # combines: DVE may read at most ONE PSUM
                                # operand per op, so ACT first copies m1 out
                                m1c = stage.tile([128, RGW, NT], F32, tag="m1c",
                                                 name="m1c", bufs=2)
                                nc.scalar.copy(out=m1c[:], in_=pts[1][:])
                                te = stage.tile([128, RGW, NT], F32, tag="te",
                                                name="te", bufs=2)
                                nc.vector.tensor_tensor(
                                    out=te[:], in0=m1c[:], in1=pts[0][:],
                                    op=ALU.add)
                                YDT = FP16 if fast else F32
                                ye = None
                                if not paired:
                                    ye = stage.tile([128, RGW, NT], YDT,
                                                    tag="ye", name="ye", bufs=2)
                                    nc.vector.tensor_tensor(
                                        out=ye[:], in0=te[:], in1=pts[2][:],
                                        op=ALU.add)
                                to = stage.tile([128, RGW, NT], F32, tag="to",
                                                name="to", bufs=2)
                                nc.vector.tensor_tensor(
                                    out=to[:], in0=m1c[:], in1=pts[2][:],
                                    op=ALU.subtract)
                                yo = None
                                if not paired:
                                    yo = stage.tile([128, RGW, NT], YDT,
                                                    tag="yo", name="yo", bufs=2)
                                    nc.vector.tensor_tensor(
                                        out=yo[:], in0=to[:], in1=pts[3][:],
                                        op=ALU.subtract)
                                if strip == "nopar":
                                    continue
                                if paired:
                                    y2 = stage.tile([128, 2, RGW, NT], F32,
                                                    tag="y2", name="y2", bufs=2)
                                    nc.vector.tensor_tensor(
                                        out=y2[:, 0], in0=te[:], in1=pts[2][:],
                                        op=ALU.add)
                                    nc.vector.tensor_tensor(
                                        out=y2[:, 1], in0=to[:], in1=pts[3][:],
                                        op=ALU.subtract)
                                    if br == 0:
                                        acc = stage.tile([128, 2, RGW, NT], F32,
                                                         tag="acc", name="acc",
                                                         bufs=5)
                                        nc.vector.tensor_scalar(
                                            out=acc[:], in0=y2[:],
                                            scalar1=cst_t[:, 0 + j:1 + j],
                                            scalar2=cst_t[:, 6:7],
                                            op0=ALU.is_gt, op1=ALU.mult)
                                        accs[r] = acc
                                    elif br == 1:
                                        t4 = stage.tile([128, 2, RGW, NT], F32,
                                                        tag="t4", name="t4", bufs=2)
                                        nc.scalar.activation(
                                            out=t4[:], in_=y2[:], func=AF.Identity,
                                            bias=cst_t[:, 2 + j:3 + j],
                                            scale=cst_t[:, 10 + j:11 + j])
                                        u4 = stage.tile([128, 2, RGW, NT], F32,
                                                        tag="u4", name="u4", bufs=2)
                                        nc.vector.tensor_scalar(
                                            out=u4[:], in0=t4[:],
                                            scalar1=float(C_MAGIC),
                                            scalar2=float(C_MAGIC + 15.0),
                                            op0=ALU.add, op1=ALU.min)
                                        q4 = stage.tile([128, 2, RGW, NT], F32,
                                                        tag="q4", name="q4", bufs=2)
                                        nc.vector.tensor_scalar(
                                            out=q4[:], in0=u4[:],
                                            scalar1=float(C_MAGIC),
                                            scalar2=float(C_MAGIC),
                                            op0=ALU.max, op1=ALU.subtract)
                                        a3 = stage.tile([128, 2, RGW, NT], F32,
                                                        tag="a3", name="a3", bufs=5)
                                        nc.vector.scalar_tensor_tensor(
                                            out=a3[:], in0=q4[:],
                                            scalar=cst_t[:, 7:8],
                                            in1=accs.pop(r)[:],
                                            op0=ALU.mult, op1=ALU.add)
                                        a3s[r] = a3
                                    else:
                                        y16 = stage.tile([128, 2, RGW, NT], F32,
                                                         tag="y16", name="y16",
                                                         bufs=2)
                                        nc.scalar.activation(
                                            out=y16[:], in_=y2[:], func=AF.Relu,
                                            bias=cst_t[:, 4 + j:5 + j], scale=1.0)
                                        v16 = stage.tile([128, 2, RGW, NT], F32,
                                                         tag="v16", name="v16",
                                                         bufs=2)
                                        nc.vector.tensor_scalar(
                                            out=v16[:], in0=y16[:],
                                            scalar1=cst_t[:, 8:9],
                                            scalar2=cst_t[:, 9:10],
                                            op0=ALU.mult, op1=ALU.min)
                                        o_full = stage.tile([128, RGW, W], F32,
                                                            tag="o", name="o",
                                                            bufs=3)
                                        ov = o_full[:].rearrange(
                                            "p r (t two) -> p two r t", two=2)
                                        nc.vector.tensor_tensor(
                                            out=ov, in0=a3s.pop(r)[:],
                                            in1=v16[:], op=ALU.add)
                                        if strip == "none":
                                            nc.sync.dma_start(
                                                out=out[img,
                                                        128 * j:128 * (j + 1),
                                                        r0:r0 + RGW, :],
                                                in_=o_full[:])
                                    continue
                                o_full = None
                                for par, y in ((0, ye), (1, yo)):
                                    if br == 0:
                                        acc = stage.tile([128, RGW, NT], YDT,
                                                         tag="acc",
                                                         name="acc", bufs=9)
                                        (nc.gpsimd if use_gpsimd else
                                         nc.vector).tensor_scalar(
                                            out=acc[:], in0=y[:],
                                            scalar1=cst_t[:, 0 + j:1 + j],
                                            scalar2=cst_t[:, 6:7],
                                            op0=ALU.is_gt, op1=ALU.mult)
                                        accs[(r, par)] = acc
                                    elif br == 1:
                                        t4 = stage.tile([128, RGW, NT], F32,
                                                        tag="t4",
                                                        name="t4", bufs=2)
                                        nc.scalar.activation(
                                            out=t4[:], in_=y[:], func=AF.Identity,
                                            bias=cst_t[:, 2 + j:3 + j],
                                            scale=cst_t[:, 10 + j:11 + j])
                                        u4 = stage.tile([128, RGW, NT], F32,
                                                        tag="u4",
                                                        name="u4", bufs=2)
                                        nc.vector.tensor_scalar(
                                            out=u4[:], in0=t4[:],
                                            scalar1=float(C_MAGIC),
                                            scalar2=float(C_MAGIC + 15.0),
                                            op0=ALU.add, op1=ALU.min)
                                        q4 = stage.tile([128, RGW, NT], F32,
                                                        tag="q4",
                                                        name="q4", bufs=2)
                                        if q4_act:
                                            # max(u,C)-C == Relu(u - C), exact
                                            nc.scalar.activation(
                                                out=q4[:], in_=u4[:],
                                                func=AF.Relu,
                                                bias=cst_t[:, 12:13], scale=1.0)
                                        else:
                                            (nc.gpsimd if use_gpsimd else
                                             nc.vector).tensor_scalar(
                                                out=q4[:], in0=u4[:],
                                                scalar1=float(C_MAGIC),
                                                scalar2=float(C_MAGIC),
                                                op0=ALU.max, op1=ALU.subtract)
                                        a3 = stage.tile([128, RGW, NT], F32,
                                                        tag="a3",
                                                        name="a3", bufs=9)
                                        nc.vector.scalar_tensor_tensor(
                                            out=a3[:], in0=q4[:],
                                            scalar=cst_t[:, 7:8],
                                            in1=accs.pop((r, par))[:],
                                            op0=ALU.mult, op1=ALU.add)
                                        a3s[(r, par)] = a3
                                    else:
                                        y16 = stage.tile([128, RGW, NT], F32,
                                                         tag="y16",
                                                         name="y16", bufs=2)
                                        nc.scalar.activation(
                                            out=y16[:], in_=y[:], func=AF.Relu,
                                            bias=cst_t[:, 4 + j:5 + j], scale=1.0)
                                        v16 = stage.tile([128, RGW, NT], F32,
                                                         tag="v16",
                                                         name="v16", bufs=2)
                                        nc.vector.tensor_scalar(
                                            out=v16[:], in0=y16[:],
                                            scalar1=cst_t[:, 8:9],
                                            scalar2=cst_t[:, 9:10],
                                            op0=ALU.mult, op1=ALU.min)
                                        if o_full is None:
                                            o_full = stage.tile(
                                                [128, RGW, W], F32,
                                                tag="o", name="o", bufs=3)
                                        (nc.gpsimd if use_gpsimd else
                                         nc.vector).tensor_tensor(
                                            out=o_full[:, :, par:56:2],
                                            in0=a3s.pop((r, par))[:],
                                            in1=v16[:], op=ALU.add)
                                        if strip == "none" and par == 1:
                                            nc.sync.dma_start(
                                                out=out[img,
                                                        128 * j:128 * (j + 1),
                                                        r0:r0 + RGW, :],
                                                in_=o_full[:])
                if loop:
                    nc.gpsimd.nop()
            if strip != "none":
                nc.sync.dma_start(out=out[0, 0:128, 0, 0:12], in_=cst_t[:, 0:12])

    nc.compile()
    return nc


def _prepare(x, Wt, bn_gamma, bn_beta, bn_mean, bn_var, alphas):
    """Host prep: exact-bf16 weights (signs / ints / folded), const vectors,
    f32r-rounded x."""
    x = np.ascontiguousarray(x, np.float32)
    Wt = np.asarray(Wt, np.float32)
    a64 = np.asarray(alphas, np.float64)
    e = np.exp(a64 - a64.max())
    wsoft = (e / e.sum()).astype(np.float64)   # [3]
    w0, w1, w2 = wsoft

    inv = (np.asarray(bn_gamma, np.float64)
           / np.sqrt(np.asarray(bn_var, np.float64) + EPS))       # [3, COUT]
    bias = (np.asarray(bn_beta, np.float64)
            - np.asarray(bn_mean, np.float64) * inv)              # [3, COUT]

    # branch weight matrices (values chosen exactly representable in bf16
    # for br0/br1; br2 carries the BN fold and rounds to bf16)
    W0 = np.sign(Wt[0]).astype(np.float64)                        # {-1,0,1}
    scale0 = np.float64(np.float32(np.mean(np.abs(Wt[0]), dtype=np.float32)))
    step1 = np.float64(np.float32(np.max(np.abs(Wt[1])) / np.float32(7.0)))
    W1 = np.round(Wt[1].astype(np.float64) / step1)               # ints [-7,7]
    step2 = np.float64(np.float32(np.max(np.abs(Wt[2])) / np.float32(32767.0)))
    Wq2 = np.round(Wt[2].astype(np.float64) / step2) * step2
    W2 = Wq2 * inv[2][:, None, None, None]
    Wf = [W0, W1, W2]

    # lhsT layout [cin_p(128), half(2), kh, kw, blk(6), cout_m(128)]
    Whost = np.empty((128, 2, 3, 3, 6, 128), np.float32)
    for i in range(3):
        for j in range(2):
            blk = 2 * i + j
            sub = Wf[i][128 * j:128 * (j + 1), :, :, :]           # [128m, 256, 3, 3]
            for h in range(2):
                Whost[:, h, :, :, blk, :] = sub[:, 128 * h:128 * (h + 1), :, :] \
                    .transpose(1, 2, 3, 0)
    wr = Whost.reshape(128, -1).astype(np.float16)

    cst = np.zeros((128, 14), np.float32)
    cst[:, 12] = -C_MAGIC
    for j in range(2):
        sl = slice(128 * j, 128 * (j + 1))
        cst[:, 0 + j] = ((0.5 - bias[0][sl]) / (scale0 * inv[0][sl])) \
            .astype(np.float32)
        cst[:, 2 + j] = (bias[1][sl] * 15.0).astype(np.float32)
        cst[:, 4 + j] = bias[2][sl].astype(np.float32)
        cst[:, 10 + j] = (step1 * inv[1][sl] * 15.0).astype(np.float32)
    cst[:, 6] = np.float32(w0)
    cst[:, 7] = np.float32(w1 / 15.0)
    cst[:, 8] = np.float32(w2)
    cst[:, 9] = np.float32(w2)

    xr = x.astype(np.float16)
    return xr, wr, cst




def _prepare_wino(x, Wt, bn_gamma, bn_beta, bn_mean, bn_var, alphas):
    """Host prep for the winograd variant: U = G g along kw (exact fp16 for
    sign/int branches), same cst vector as the direct variant."""
    x = np.ascontiguousarray(x, np.float32)
    Wt = np.asarray(Wt, np.float32)
    _, wr_direct, cst = _prepare(x, Wt, bn_gamma, bn_beta, bn_mean, bn_var,
                                 alphas)

    inv = (np.asarray(bn_gamma, np.float64)
           / np.sqrt(np.asarray(bn_var, np.float64) + EPS))
    W0 = np.sign(Wt[0]).astype(np.float64)
    step1 = np.float64(np.float32(np.max(np.abs(Wt[1])) / np.float32(7.0)))
    W1 = np.round(Wt[1].astype(np.float64) / step1)
    step2 = np.float64(np.float32(np.max(np.abs(Wt[2])) / np.float32(32767.0)))
    Wq2 = np.round(Wt[2].astype(np.float64) / step2) * step2
    W2 = Wq2 * inv[2][:, None, None, None]
    Wf = [W0, W1, W2]

    # U layout [cin_p(128), half(2), point(4), kh(3), blk(6), cout_m(128)]
    Uhost = np.empty((128, 2, 4, 3, 6, 128), np.float32)
    for i in range(3):
        g = Wf[i]                                     # [co, ci, kh, kw]
        U = np.stack([g[:, :, :, 0],
                      (g[:, :, :, 0] + g[:, :, :, 1] + g[:, :, :, 2]) / 2,
                      (g[:, :, :, 0] - g[:, :, :, 1] + g[:, :, :, 2]) / 2,
                      g[:, :, :, 2]], axis=2)         # [co, ci, 4p, 3kh]
        for j in range(2):
            blk = 2 * i + j
            sub = U[128 * j:128 * (j + 1)]            # [128co, 256ci, 4, 3]
            for h in range(2):
                Uhost[:, h, :, :, blk, :] = \
                    sub[:, 128 * h:128 * (h + 1)].transpose(1, 2, 3, 0)
    wr = Uhost.reshape(128, -1).astype(np.float16)
    xr = x.astype(np.float16)
    return xr, wr, cst


def kernel(x, W, bn_gamma, bn_beta, bn_mean, bn_var, alphas,
           _iters=1, variant="winov", _strip="none"):
    loop = _iters > 1
    key = (variant, loop, _strip)
    if key not in _cache:
        if variant == "flat":
            _cache[key] = _build_flat(loop=loop, strip=_strip)
        elif variant == "wino":
            _cache[key] = _build_wino(loop=loop, strip=_strip)
        elif variant == "winov":
            _cache[key] = _build_wino(loop=loop, strip=_strip,
                                      use_gpsimd=False)
        elif variant == "winop":
            _cache[key] = _build_wino(loop=loop, strip=_strip,
                                      use_gpsimd=False, paired=True)
        elif variant == "winof":
            _cache[key] = _build_wino(loop=loop, strip=_strip,
                                      use_gpsimd=False, fast=True)
        elif variant == "winoq":
            _cache[key] = _build_wino(loop=loop, strip=_strip,
                                      use_gpsimd=False, q4_act=True)
        else:
            _cache[key] = _build(variant, loop=loop, strip=_strip)
    nc = _cache[key]
    prep = _prepare_wino if variant in ("wino", "winov", "winop",
                                        "winof", "winoq") else _prepare
    xr, wr, cst = prep(x, W, bn_gamma, bn_beta, bn_mean, bn_var, alphas)

    it = np.array([[_iters]], np.int32)
    in_maps = [
        {"xr": xr[B_PER * c:B_PER * (c + 1)], "wr": wr, "cst": cst, "iters": it}
        for c in range(N_CORES)
    ]
    res = run_bass_kernel_spmd(nc, in_maps, list(range(N_CORES)))
    outs = [res.results[c]["out"] for c in range(N_CORES)]
    return np.concatenate(outs, axis=0)
